# revision 4
# baseline (speedup 1.0000x reference)
"""Trainium2 Bass kernel for the 2-qubit EstimatorQNN forward pass.

The circuit collapses analytically to THREE cosines (out = u^T K v in the
basis u=(1,cos pi x0,sin pi x0), v=(1,cos pi x1,sin pi x1); K row 0 is zero
and the 2x2 trailing block is rank-1 — both asserted at derive time):

  out = Rk*cos(pi x0 - phk) + Rw*Rz*cos(pi x0 - phw)*cos(pi x1 - phz)

Each cosine costs ONE custom-DVE ADD_RANGE_WRAP + a share of ONE Sin
activation:

  cos(pi(x+d)) = sin(pi(x+e)),  e = d+1/2 in (-1/2, 3/2]
               = sin(pi*w),     w = arw(x; shift=e, bound=1, period=2)

arw wraps x+e into [-1,1] by one +-2 step, so the Sin argument pi*w is in
the ScalarE-valid [-pi,pi] and the sign is POSITIVE cosine directly.

Device op graph per tile (x0,x1 fp16, de-interleaved host-side):
  DVE : w0=arw(x0,e0)  w1=arw(x0,e1)  w2=arw(x1,e2)   (custom DVE)
  ACT : T = sin(pi*W) over [w0|w1|w2]                 (ONE Sin instruction)
  POOL: ct=p*t_small ; m=t_a*t2 ; y=m+t_b             (ts + tt + tt)
Host scales by S = max(Rk, Rw*Rz); p = min/max <= 1.

Everything is fp16 in SBUF and over DMA (in 2.1MB + out 1.05MB per core,
halving the f32 HBM traffic). The emission order is software-pipelined
(pre-ops of tile t+2 issue before post-ops of tile t) so the in-order
DVE/Pool queues never starve the ACT queue.

The 4 weight-dependent constants are baked as instruction immediates, so
the program is rebuilt per distinct weight vector; the neuronxcc disk
cache makes repeat compiles for the same weights instant.
"""

import sys

if "/opt/trn_rl_repo" not in sys.path:
    sys.path.insert(0, "/opt/trn_rl_repo")

import numpy as np

import concourse.bass as bass
import concourse.bacc as bacc
import concourse.mybir as mybir
import concourse.tile as tile
from concourse.bass_utils import run_bass_kernel_spmd

N_CORES = 8
B = 4194304
BC = B // N_CORES            # samples per core
P = 128                      # SBUF partitions
H = 1024                     # samples per partition-row per tile
NT = BC // (P * H)           # 4 tiles

F16 = mybir.dt.float16
F32 = mybir.dt.float32
PI = float(np.float64(np.pi))

_N_QUBITS, _N_LAYERS = 2, 2


# ----------------------------------------------------------------- host math

def _circuit_unitary(w):
    """Fixed 4x4 unitary of the variational layers (float64 complex)."""
    def rx(t):
        c, s = np.cos(t / 2), np.sin(t / 2)
        return np.array([[c, -1j * s], [-1j * s, c]])

    def rz(t):
        c, s = np.cos(t / 2), np.sin(t / 2)
        return np.array([[c - 1j * s, 0], [0, c + 1j * s]])

    def ry(t):
        c, s = np.cos(t / 2), np.sin(t / 2)
        return np.array([[c, -s], [s, c]])

    I2 = np.eye(2)
    CNOT = np.array(
        [[1, 0, 0, 0], [0, 1, 0, 0], [0, 0, 0, 1], [0, 0, 1, 0]], dtype=complex
    )
    U = np.eye(4, dtype=complex)
    off = 0
    for _ in range(_N_LAYERS):
        for q in range(_N_QUBITS):
            for G in (
                rx(w[off + q * 3 + 0]),
                rz(w[off + q * 3 + 1]),
                ry(w[off + q * 3 + 2]),
            ):
                M = np.kron(G, I2) if q == 0 else np.kron(I2, G)
                U = M @ U
        U = CNOT @ U
        off += _N_QUBITS * 3
    return U


def _derive_consts(weights):
    """weights[12] -> (e0, e1, e2, p, case_b, S).

    out = Rk*cos(pi x0 - phk) + Rw*Rz*cos(pi x0 - phw)*cos(pi x1 - phz)
    case_b = (Rw*Rz >= Rk); p = min/max of the two amplitudes; S = max.
    e_j in (-1/2, 3/2] are the arw shifts (see module docstring).
    """
    w = np.asarray(weights, dtype=np.float64)
    U = _circuit_unitary(w)
    Z0 = np.diag([1.0, 1.0, -1.0, -1.0])
    A = np.real(U.conj().T @ Z0 @ U)

    I2 = np.eye(2)
    Z = np.diag([1.0, -1.0])
    X = np.array([[0.0, 1.0], [1.0, 0.0]])
    Pb = [I2, Z, X]
    K = np.zeros((3, 3))
    for p_ in range(3):
        for q_ in range(3):
            K[p_, q_] = 0.25 * sum(
                A[2 * i + j, 2 * k + l] * Pb[p_][i, k] * Pb[q_][j, l]
                for i in range(2)
                for j in range(2)
                for k in range(2)
                for l in range(2)
            )

    scale = max(np.abs(K).max(), 1e-30)
    assert np.abs(K[0]).max() < 1e-9 * scale, (
        f"structure violated: K row0 nonzero ({K[0]})"
    )

    K10, K20 = K[1, 0], K[2, 0]
    M = K[1:, 1:]
    u_, s_, vt_ = np.linalg.svd(M)
    assert s_[1] < 1e-9 * scale, f"structure violated: rank-1 residual {s_}"
    wvec = u_[:, 0] * s_[0]
    zvec = vt_[0, :]

    Rk, phk = float(np.hypot(K10, K20)), float(np.arctan2(K20, K10))
    Rw, phw = float(np.hypot(*wvec)), float(np.arctan2(wvec[1], wvec[0]))
    Rz, phz = float(np.hypot(*zvec)), float(np.arctan2(zvec[1], zvec[0]))
    Rwz = Rw * Rz

    def efold(phi):
        # cos(pi x - phi) = sin(pi(x + e)), e = 1/2 - phi/pi, d in (-1,1]
        d = -phi / np.pi
        d = d - 2 * np.floor((d + 1) / 2)  # (-1, 1]
        return float(d + 0.5)              # (-1/2, 3/2]

    S = max(Rk, Rwz)
    if S < 1e-30:
        return (0.5, 0.5, 0.5, 0.0, True, 0.0)
    case_b = Rwz >= Rk
    p = (Rk / Rwz) if case_b else (Rwz / Rk)
    return (efold(phk), efold(phw), efold(phz), float(p), bool(case_b), float(S))


# ------------------------------------------------------------- device program

def build_program(consts, nt=NT, h=H):
    """Build the per-core Bass program with the constants as immediates."""
    e0, e1, e2, p, case_b = consts[0], consts[1], consts[2], consts[3], consts[4]
    e0, e1, e2, p = (float(np.float32(v)) for v in (e0, e1, e2, p))

    nc = bacc.Bacc("TRN2", target_bir_lowering=False, debug=False)

    # fp16 zero const AP for the Sin bias
    zero16 = nc.alloc_sbuf_tensor("const-zero16", [P, 1], F16)
    nc.gpsimd.memset(zero16.ap(), 0.0)
    nc.all_engine_barrier()

    # per-tile input slab [x0 | x1] and one output tensor per tile (avoids
    # whole-tensor WAW ordering between the out-DMAs; the DMA instruction
    # encoding only fits one sync wait)
    xin = nc.dram_tensor("xin", [nt, P, 2 * h], F16, kind="ExternalInput")
    ys = [
        nc.dram_tensor(f"y{t}", [P, h], F16, kind="ExternalOutput")
        for t in range(nt)
    ]

    SIN = mybir.ActivationFunctionType.Sin
    MULT = mybir.AluOpType.mult
    ADD = mybir.AluOpType.add

    with tile.TileContext(nc) as tc:
        with (
            tc.tile_pool(name="xpool", bufs=3) as xpool,
            tc.tile_pool(name="wpool", bufs=3) as wpool,
            tc.tile_pool(name="tpool", bufs=3) as tpool,
            tc.tile_pool(name="ppool", bufs=3) as ppool,
            tc.tile_pool(name="mpool", bufs=3) as mpool,
            tc.tile_pool(name="opool", bufs=3) as opool,
        ):
            Ts = {}

            def pre(t):
                """DMA in + arw folds + the single Sin instruction."""
                X = xpool.tile([P, 2 * h], F16, tag="x")
                nc.sync.dma_start(X[:, 0:h], xin[t, :, 0:h])
                nc.sync.dma_start(X[:, h:2 * h], xin[t, :, h:2 * h])
                W = wpool.tile([P, 3 * h], F16, tag="w")
                nc.vector.add_range_wrap(W[:, 0:h], X[:, 0:h], e0, 1.0, 2.0)
                nc.vector.add_range_wrap(W[:, h:2 * h], X[:, 0:h], e1, 1.0, 2.0)
                nc.vector.add_range_wrap(
                    W[:, 2 * h:3 * h], X[:, h:2 * h], e2, 1.0, 2.0
                )
                T = tpool.tile([P, 3 * h], F16, tag="t")
                # t_j = sin(pi*w_j) = +cos_j
                nc.scalar.activation(T[:], W[:], SIN, bias=zero16.ap(), scale=PI)
                Ts[t] = T

            def post(t):
                """Combine on Pool + DMA out.
                case B (S=Rwz): y = t1*t2 + p*t0
                case A (S=Rk):  y = (p*t1)*t2 + t0
                Host multiplies by S."""
                T = Ts.pop(t)
                t0, t1, t2 = T[:, 0:h], T[:, h:2 * h], T[:, 2 * h:3 * h]
                CT = ppool.tile([P, h], F16, tag="ct")
                M = mpool.tile([P, h], F16, tag="m")
                Y = opool.tile([P, h], F16, tag="y")
                if case_b:
                    nc.gpsimd.tensor_scalar(CT[:], t0, p, None, MULT)
                    nc.gpsimd.tensor_tensor(M[:], t1, t2, MULT)
                    nc.gpsimd.tensor_tensor(Y[:], M[:], CT[:], ADD)
                else:
                    nc.gpsimd.tensor_scalar(CT[:], t1, p, None, MULT)
                    nc.gpsimd.tensor_tensor(M[:], CT[:], t2, MULT)
                    nc.gpsimd.tensor_tensor(Y[:], M[:], t0, ADD)
                nc.sync.dma_start(ys[t][:], Y[:])

            pre(0)
            if nt > 1:
                pre(1)
            for t in range(nt):
                post(t)
                if t + 2 < nt:
                    pre(t + 2)

    nc.compile()
    return nc


_PROGRAM_CACHE = {}


def _get_program(consts, nt=NT, h=H):
    key = (
        tuple(float(np.float32(v)) for v in consts[:4]),
        bool(consts[4]),
        nt,
        h,
    )
    if key not in _PROGRAM_CACHE:
        _PROGRAM_CACHE[key] = build_program(consts, nt, h)
    return _PROGRAM_CACHE[key]


def make_in_maps(inputs, nt=NT, h=H, n_cores=N_CORES):
    """Shard full inputs into per-core fp16 input maps (host de-interleave)."""
    x = np.asarray(inputs)
    xh = x.astype(np.float16)
    x0 = xh[:, 0].reshape(n_cores, nt, P, h)
    x1 = xh[:, 1].reshape(n_cores, nt, P, h)
    xin = np.concatenate([x0, x1], axis=-1)  # [cores, nt, P, 2h]
    return [{"xin": xin[i]} for i in range(n_cores)]


def kernel(inputs, weights):
    """Full inputs in, full output out (see module docstring)."""
    consts = _derive_consts(weights)
    nc = _get_program(consts)
    in_maps = make_in_maps(inputs)
    res = run_bass_kernel_spmd(nc, in_maps, list(range(N_CORES)))
    out = np.concatenate(
        [
            np.asarray(r[f"y{t}"], dtype=np.float32).reshape(-1)
            for r in res.results
            for t in range(NT)
        ]
    )
    return (np.float32(consts[5]) * out).reshape(B, 1).astype(np.float32)


# revision 5
# speedup vs baseline: 2.7294x; 2.7294x over previous
"""Trainium2 Bass kernel for the 2-qubit EstimatorQNN forward pass.

The circuit collapses analytically to THREE cosines (out = u^T K v in the
basis u=(1,cos pi x0,sin pi x0), v=(1,cos pi x1,sin pi x1); K row 0 is zero
and the 2x2 trailing block is rank-1 — both asserted at derive time):

  out = Rk*cos(pi x0 - phk) + Rw*Rz*cos(pi x0 - phw)*cos(pi x1 - phz)

Measured engine realities on TRN2 drive the op placement (HW trace):
ACT any-activation ~0.93ns/elem (no fp16 speedup); DVE fp16 tensor_scalar
0.32ns (4x), tensor_tensor 0.58ns (2x), stt/custom 1.1ns (1x); Pool
(gpsimd) ~2.05ns/elem and cross-tile ADD is pathological (~14ns/elem) —
so Pool gets exactly one same-tile multiply.  The Sin table is valid on
[-pi,pi] ONLY (verified on HW).

Per-element math (device ships TWO streams, host does the 2-term combine):
  t0  = sin(pi(x0+e0+m0))            m0 in {0,-+2}: one-sided range wrap
  t1  = sin(pi(x0+e1+m1))            via is_gt/is_lt mask (DVE ts) + add
  t2' = sin((pi/2)x1 + psi)          half-angle, in-range, NO wrap
  M2  = t1*(1-2*t2'^2)               = t1*cos(pi x1 - phz)
  host: out = S*(ca*t0 + cb*M2)      (ca,cb) = (p,1) or (1,p)

The phase shifts e_j ride in the per-slot Sin activation bias APs (f32),
so the DVE wrap only adds the {0,-+2} correction.  Per tile:
  DVE : m0,m1 (ts is_gt/lt*-+2)  w0,w1 (tt add)  V=1-2G (ts)  M2=t1*V (tt)
  ACT : three Sin instructions, t2' FIRST (frees the Pool/DVE post chain)
  POOL: G = t2'*t2' (same-tile tensor_tensor mult)
fp16 everywhere in SBUF and over DMA (in 2.1MB + out 2.1MB per core).
Emission is software-pipelined (pre of tile t+2 before post of tile t).

Only (e0, e1, psi) are baked into the program; amplitude ratio p, scale S
and the A/B case live in the host combine, so one compiled program serves
any weights with the same phases (and the neuronxcc disk cache makes
repeat compiles instant).
"""

import sys

if "/opt/trn_rl_repo" not in sys.path:
    sys.path.insert(0, "/opt/trn_rl_repo")

import numpy as np

import concourse.bass as bass
import concourse.bacc as bacc
import concourse.mybir as mybir
import concourse.tile as tile
from concourse.bass_utils import run_bass_kernel_spmd

N_CORES = 8
B = 4194304
BC = B // N_CORES            # samples per core
P = 128                      # SBUF partitions
H = 1024                     # samples per partition-row per tile
NT = BC // (P * H)           # 4 tiles

F16 = mybir.dt.float16
F32 = mybir.dt.float32
PI = float(np.float64(np.pi))

_N_QUBITS, _N_LAYERS = 2, 2


# ----------------------------------------------------------------- host math

def _circuit_unitary(w):
    """Fixed 4x4 unitary of the variational layers (float64 complex)."""
    def rx(t):
        c, s = np.cos(t / 2), np.sin(t / 2)
        return np.array([[c, -1j * s], [-1j * s, c]])

    def rz(t):
        c, s = np.cos(t / 2), np.sin(t / 2)
        return np.array([[c - 1j * s, 0], [0, c + 1j * s]])

    def ry(t):
        c, s = np.cos(t / 2), np.sin(t / 2)
        return np.array([[c, -s], [s, c]])

    I2 = np.eye(2)
    CNOT = np.array(
        [[1, 0, 0, 0], [0, 1, 0, 0], [0, 0, 0, 1], [0, 0, 1, 0]], dtype=complex
    )
    U = np.eye(4, dtype=complex)
    off = 0
    for _ in range(_N_LAYERS):
        for q in range(_N_QUBITS):
            for G in (
                rx(w[off + q * 3 + 0]),
                rz(w[off + q * 3 + 1]),
                ry(w[off + q * 3 + 2]),
            ):
                M = np.kron(G, I2) if q == 0 else np.kron(I2, G)
                U = M @ U
        U = CNOT @ U
        off += _N_QUBITS * 3
    return U


def _derive_consts(weights):
    """weights[12] -> (e0, e1, psi, p, case_b, S).

    out = Rk*cos(pi x0 - phk) + Rw*Rz*cos(pi x0 - phw)*cos(pi x1 - phz)
    e0, e1 in (-1/2, 3/2]: sin-form shifts for the two x0 cosines.
    psi = -phz/2: half-angle bias for the x1 cosine.
    case_b = (Rw*Rz >= Rk); p = min/max of the amplitudes; S = max.
    """
    w = np.asarray(weights, dtype=np.float64)
    U = _circuit_unitary(w)
    Z0 = np.diag([1.0, 1.0, -1.0, -1.0])
    A = np.real(U.conj().T @ Z0 @ U)

    I2 = np.eye(2)
    Z = np.diag([1.0, -1.0])
    X = np.array([[0.0, 1.0], [1.0, 0.0]])
    Pb = [I2, Z, X]
    K = np.zeros((3, 3))
    for p_ in range(3):
        for q_ in range(3):
            K[p_, q_] = 0.25 * sum(
                A[2 * i + j, 2 * k + l] * Pb[p_][i, k] * Pb[q_][j, l]
                for i in range(2)
                for j in range(2)
                for k in range(2)
                for l in range(2)
            )

    scale = max(np.abs(K).max(), 1e-30)
    assert np.abs(K[0]).max() < 1e-9 * scale, (
        f"structure violated: K row0 nonzero ({K[0]})"
    )

    K10, K20 = K[1, 0], K[2, 0]
    M = K[1:, 1:]
    u_, s_, vt_ = np.linalg.svd(M)
    assert s_[1] < 1e-9 * scale, f"structure violated: rank-1 residual {s_}"
    wvec = u_[:, 0] * s_[0]
    zvec = vt_[0, :]

    Rk, phk = float(np.hypot(K10, K20)), float(np.arctan2(K20, K10))
    Rw, phw = float(np.hypot(*wvec)), float(np.arctan2(wvec[1], wvec[0]))
    Rz, phz = float(np.hypot(*zvec)), float(np.arctan2(zvec[1], zvec[0]))
    Rwz = Rw * Rz

    def efold(phi):
        # cos(pi x - phi) = sin(pi(x + e)), e = 1/2 - phi/pi, d in (-1,1]
        d = -phi / np.pi
        d = d - 2 * np.floor((d + 1) / 2)  # (-1, 1]
        return float(d + 0.5)              # (-1/2, 3/2]

    S = max(Rk, Rwz)
    if S < 1e-30:
        return (0.5, 0.5, 0.0, 0.0, True, 0.0)
    case_b = Rwz >= Rk
    p = (Rk / Rwz) if case_b else (Rwz / Rk)
    return (efold(phk), efold(phw), float(-phz / 2), float(p), bool(case_b),
            float(S))


# ------------------------------------------------------------- device program

def build_program(consts, nt=NT, h=H):
    """Per-core Bass program; only (e0, e1, psi) are baked in."""
    e0, e1, psi = (float(np.float32(v)) for v in consts[:3])

    nc = bacc.Bacc("TRN2", target_bir_lowering=False, debug=False)

    # f32 per-slot Sin bias APs
    biases = []
    for i, bval in enumerate((PI * e0, PI * e1, psi)):
        t = nc.alloc_sbuf_tensor(f"const-b{i}", [P, 1], F32)
        nc.gpsimd.memset(t.ap(), bval)
        biases.append(t)
    nc.all_engine_barrier()

    xin = nc.dram_tensor("xin", [nt, P, 2 * h], F16, kind="ExternalInput")
    # separate out tensors per tile/stream: the DMA instruction encoding
    # only fits one sync wait, and whole-tensor WAW ordering would
    # serialize the out-DMAs
    y_m2 = [
        nc.dram_tensor(f"m2_{t}", [P, h], F16, kind="ExternalOutput")
        for t in range(nt)
    ]
    y_t0 = [
        nc.dram_tensor(f"t0_{t}", [P, h], F16, kind="ExternalOutput")
        for t in range(nt)
    ]

    SIN = mybir.ActivationFunctionType.Sin
    MULT = mybir.AluOpType.mult
    ADD = mybir.AluOpType.add
    ISGT = mybir.AluOpType.is_gt
    ISLT = mybir.AluOpType.is_lt

    def mask_args(e):
        # one-sided wrap of x+e into [-1,1]: m in {0, -+2}
        if e > 0:
            return (float(np.float32(1.0 - e)), -2.0, ISGT)
        return (float(np.float32(-1.0 - e)), 2.0, ISLT)

    thr0, per0, cmp0 = mask_args(e0)
    thr1, per1, cmp1 = mask_args(e1)

    with tile.TileContext(nc) as tc:
        with (
            tc.tile_pool(name="xpool", bufs=3) as xpool,
            tc.tile_pool(name="kpool", bufs=3) as kpool,
            tc.tile_pool(name="wpool", bufs=3) as wpool,
            tc.tile_pool(name="tpool", bufs=3) as tpool,
            tc.tile_pool(name="gpool", bufs=3) as gpool,
            tc.tile_pool(name="vpool", bufs=3) as vpool,
            tc.tile_pool(name="mpool", bufs=3) as mpool,
        ):
            Ts = {}

            def pre(t):
                X = xpool.tile([P, 2 * h], F16, tag="x")
                X0, X1 = X[:, 0:h], X[:, h:2 * h]
                nc.sync.dma_start(X0, xin[t, :, 0:h])
                nc.sync.dma_start(X1, xin[t, :, h:2 * h])
                MK = kpool.tile([P, 2 * h], F16, tag="mk")
                W = wpool.tile([P, 2 * h], F16, tag="w")
                T = tpool.tile([P, 3 * h], F16, tag="t")
                # t2' first: it only needs the X1 DMA, and the Pool/DVE
                # post chain hangs off it
                nc.scalar.activation(
                    T[:, 2 * h:3 * h], X1, SIN, bias=biases[2].ap(), scale=PI / 2
                )
                nc.vector.tensor_scalar(MK[:, 0:h], X0, thr0, per0, cmp0, MULT)
                nc.vector.tensor_scalar(MK[:, h:2 * h], X0, thr1, per1, cmp1, MULT)
                nc.vector.tensor_tensor(W[:, 0:h], X0, MK[:, 0:h], ADD)
                nc.vector.tensor_tensor(W[:, h:2 * h], X0, MK[:, h:2 * h], ADD)
                nc.scalar.activation(
                    T[:, h:2 * h], W[:, h:2 * h], SIN, bias=biases[1].ap(), scale=PI
                )
                nc.scalar.activation(
                    T[:, 0:h], W[:, 0:h], SIN, bias=biases[0].ap(), scale=PI
                )
                Ts[t] = T

            def post(t):
                T = Ts.pop(t)
                t0, t1, t2 = T[:, 0:h], T[:, h:2 * h], T[:, 2 * h:3 * h]
                G = gpool.tile([P, h], F16, tag="g")
                V = vpool.tile([P, h], F16, tag="v")
                M2 = mpool.tile([P, h], F16, tag="m2")
                nc.gpsimd.tensor_tensor(G[:], t2, t2, MULT)      # same-tile mult
                nc.vector.tensor_scalar(V[:], G[:], -2.0, 1.0, MULT, ADD)
                nc.vector.tensor_tensor(M2[:], t1, V[:], MULT)
                nc.sync.dma_start(y_m2[t][:], M2[:])
                nc.sync.dma_start(y_t0[t][:], t0)

            pre(0)
            if nt > 1:
                pre(1)
            for t in range(nt):
                post(t)
                if t + 2 < nt:
                    pre(t + 2)

    nc.compile()
    return nc


_PROGRAM_CACHE = {}


def _get_program(consts, nt=NT, h=H):
    key = (tuple(float(np.float32(v)) for v in consts[:3]), nt, h)
    if key not in _PROGRAM_CACHE:
        _PROGRAM_CACHE[key] = build_program(consts, nt, h)
    return _PROGRAM_CACHE[key]


def make_in_maps(inputs, nt=NT, h=H, n_cores=N_CORES):
    """Shard full inputs into per-core fp16 input maps (host de-interleave)."""
    x = np.asarray(inputs)
    xh = x.astype(np.float16)
    x0 = xh[:, 0].reshape(n_cores, nt, P, h)
    x1 = xh[:, 1].reshape(n_cores, nt, P, h)
    xin = np.concatenate([x0, x1], axis=-1)  # [cores, nt, P, 2h]
    return [{"xin": xin[i]} for i in range(n_cores)]


def kernel(inputs, weights):
    """Full inputs in, full output out (see module docstring)."""
    consts = _derive_consts(weights)
    nc = _get_program(consts)
    in_maps = make_in_maps(inputs)
    res = run_bass_kernel_spmd(nc, in_maps, list(range(N_CORES)))
    p, case_b, S = consts[3], consts[4], consts[5]
    ca, cb = (p, 1.0) if case_b else (1.0, p)
    t0 = np.concatenate(
        [
            np.asarray(r[f"t0_{t}"], dtype=np.float32).reshape(-1)
            for r in res.results
            for t in range(NT)
        ]
    )
    m2 = np.concatenate(
        [
            np.asarray(r[f"m2_{t}"], dtype=np.float32).reshape(-1)
            for r in res.results
            for t in range(NT)
        ]
    )
    out = np.float32(S) * (np.float32(ca) * t0 + np.float32(cb) * m2)
    return out.reshape(B, 1).astype(np.float32)
